# revision 21
# baseline (speedup 1.0000x reference)
"""Trainium2 Bass kernel for the Dynamic MultiTeacher distillation loss.

Data-parallel over 8 NeuronCores, 1024 rows each, 8 row-tiles of 128.

Per tile [128, 1000] (all f32; PE inputs viewed as float32r):
  - PE (f32r identity matmuls; ~3.4x faster than fp32, rel err ~1.7e-4):
      d_0, d_1 = x_t - s                     (PSUM)
      psq      = 0.25*(x1+x2+x3+x4)          (PSUM, = mimic logits)
      dmq      = 0.25*(x1+x2+x3+x4) - s      (PSUM, = psq - s)
  - Pool: d_2, d_3 = x_t - s (SBUF out; Pool cannot touch PSUM and must not
    be loaded much beyond this - it is slow (~2.5us/pass) and extra work on
    it stalls the DVE dots that consume d_2/d_3).
  - ScalarE (7 exp passes, 1 elem/cycle): e_t = exp(x_t/20) + accum S_t
    (x4), e_m = exp(psq/20) + S_m, exp(s) + S1, exp(s/20) + S20 (sinks).
    Teacher exps first so DVE dots aren't starved; sinks last. Also 2 small
    Copy+accum reduces for the dot tails below.
  - VectorE (bottleneck, 10 passes): max8 x5 (4 teachers + mimic) and 5
    fused dots D_t = sum(e_t * d_t), D_m = sum(e_m * dmq) via
    scalar_tensor_tensor with full-tensor out (faster than broadcast out).
    The dots for t=2,3 cover only cols [0:576] on DVE; their tails are a
    Pool elementwise multiply + ScalarE Copy-with-accum, rebalancing ~1.5
    DVE passes onto the two less-loaded engines.
  - All input DMAs ride the sync queue (HWDGE triggers on Pool/Scalar cost
    those engines ~0.6us each).

Host (tiny O(B) work in f64): gathers x_t[i,t_i], global min/max scalars,
margins = relu(gathered - m2), KD_t = T*D_t/S_t + T^2*(lse_s - lse_t),
CE = lse1 - s_gathered, threshold softmax, w1/w2 blend, mean.
"""

import numpy as np

N_CORES = 8
B_FULL = 8192
C_DIM = 1000
B_LOC = B_FULL // N_CORES          # 1024 rows per core
P = 128                            # partitions
N_TILES = B_LOC // P               # 8 row-tiles per core

T_KD = 20.0
T_THR = 6.0
EPS = 1e-05

# device output column layout: [P, 52]
#   cols 8t..8t+7 : top8 of teacher t (t=0..3 real, t=4 mimic psq)
#   col 40+t      : S_t = sum exp(x_t/20)   (t=4 mimic)
#   col 45        : S1  = sum exp(s)
#   col 46        : S20 = sum exp(s/20)
#   col 47+t      : D_t = sum e_t*(x_t - s) (t=4: D_m = sum e_m*dmq)
#   col 52/53     : partial dot tails for t=2/3 over cols 704:1000
OUT_COLS = 54

_CACHE = {}


def _build_nc():
    import concourse.bacc as bacc
    import concourse.mybir as mybir
    from concourse import tile

    nc = bacc.Bacc(
        "TRN2",
        target_bir_lowering=False,
        debug=False,
        num_devices=N_CORES,
    )
    f32 = mybir.dt.float32
    f32r = mybir.dt.float32r
    Alu = mybir.AluOpType
    Act = mybir.ActivationFunctionType

    xs = [
        nc.dram_tensor(f"x{t}", [B_LOC, C_DIM], f32, kind="ExternalInput").ap()
        for t in range(4)
    ]
    s_dram = nc.dram_tensor("s", [B_LOC, C_DIM], f32, kind="ExternalInput").ap()
    ident = nc.dram_tensor("ident", [P, P], f32, kind="ExternalInput").ap()
    negid = nc.dram_tensor("negid", [P, P], f32, kind="ExternalInput").ap()
    quarter = nc.dram_tensor("quarter", [P, P], f32, kind="ExternalInput").ap()
    res = nc.dram_tensor("res", [B_LOC, OUT_COLS], f32, kind="ExternalOutput").ap()

    HALVES = ((0, 512), (512, C_DIM))
    CSPL = 640
    CSPL_N = C_DIM - CSPL

    with tile.TileContext(nc) as tc:
        with (
            tc.tile_pool(name="const", bufs=1) as cpool,
            tc.tile_pool(name="io", bufs=3) as xpool,
            tc.tile_pool(name="exps", bufs=2) as epool,
            tc.tile_pool(name="pdif", bufs=2) as pdpool,
            tc.tile_pool(name="sink", bufs=2) as spool,
            tc.tile_pool(name="sc", bufs=2) as scpool,
            tc.tile_pool(name="outs", bufs=2) as opool,
            tc.tile_pool(name="psd", bufs=1, space="PSUM") as psdpool,
            tc.tile_pool(name="psq", bufs=1, space="PSUM") as psqpool,
            tc.tile_pool(name="psm", bufs=1, space="PSUM") as psmpool,
        ):
            # f32r weight tiles; triggered from the scalar queue so the
            # sync/gpsimd queues start streaming x/s immediately
            id_t = cpool.tile([P, P], f32r, tag="id")
            nc.scalar.dma_start(out=id_t[:], in_=ident.bitcast(f32r))
            nid_t = cpool.tile([P, P], f32r, tag="nid")
            nc.scalar.dma_start(out=nid_t[:], in_=negid.bitcast(f32r))
            q_t = cpool.tile([P, P], f32r, tag="q")
            nc.scalar.dma_start(out=q_t[:], in_=quarter.bitcast(f32r))

            for i in range(N_TILES):
                r0 = i * P
                st = xpool.tile([P, C_DIM], f32, tag="s")
                xt_tiles = [None] * 4

                def load_x(t, eng):
                    xt = xpool.tile([P, C_DIM], f32, tag=f"x{t}")
                    eng.dma_start(
                        out=xt[:].bitcast(f32r),
                        in_=xs[t][r0 : r0 + P, :].bitcast(f32r),
                    )
                    xt_tiles[t] = xt

                def load_s(eng):
                    eng.dma_start(
                        out=st[:].bitcast(f32r),
                        in_=s_dram[r0 : r0 + P, :].bitcast(f32r),
                    )

                if i == 0:
                    # cold start: x0 first so the first max8/exp unblock
                    # ASAP; x2/x3 ride the idle gpsimd queue in parallel
                    load_x(0, nc.sync)
                    load_s(nc.sync)
                    load_x(1, nc.sync)
                    load_x(2, nc.gpsimd)
                    load_x(3, nc.gpsimd)
                else:
                    load_s(nc.sync)
                    for t in range(4):
                        load_x(t, nc.sync)

                out_t = opool.tile([P, OUT_COLS], f32)

                # --- PE: d0, d1 (PSUM), psq, dmq ---
                dts = [None] * 4
                for t in (0, 1):
                    dt = psdpool.tile([P, C_DIM], f32, tag=f"d{t}")
                    for c0, c1 in HALVES:
                        nc.tensor.matmul(
                            dt[:, c0:c1], id_t[:],
                            xt_tiles[t][:, c0:c1].bitcast(f32r),
                            start=True, stop=False,
                        )
                        nc.tensor.matmul(
                            dt[:, c0:c1], nid_t[:],
                            st[:, c0:c1].bitcast(f32r),
                            start=False, stop=True,
                        )
                    dts[t] = dt

                psq = psqpool.tile([P, C_DIM], f32, tag="q")
                for c0, c1 in HALVES:
                    for t in range(4):
                        nc.tensor.matmul(
                            psq[:, c0:c1], q_t[:],
                            xt_tiles[t][:, c0:c1].bitcast(f32r),
                            start=(t == 0), stop=(t == 3),
                        )
                dmq = psmpool.tile([P, C_DIM], f32, tag="m")
                for c0, c1 in HALVES:
                    for t in range(4):
                        nc.tensor.matmul(
                            dmq[:, c0:c1], q_t[:],
                            xt_tiles[t][:, c0:c1].bitcast(f32r),
                            start=(t == 0), stop=False,
                        )
                    nc.tensor.matmul(
                        dmq[:, c0:c1], nid_t[:],
                        st[:, c0:c1].bitcast(f32r),
                        start=False, stop=True,
                    )

                # --- Pool: d2, d3 (SBUF out) ---
                for t in (2, 3):
                    dt = pdpool.tile([P, C_DIM], f32, tag=f"d{t}")
                    nc.gpsimd.tensor_tensor(
                        out=dt[:], in0=xt_tiles[t][:], in1=st[:],
                        op=Alu.subtract,
                    )
                    dts[t] = dt

                # --- ACT: teacher exps first, mimic, sinks last ---
                ets = []
                for t in range(4):
                    et = epool.tile([P, C_DIM], f32, tag=f"e{t % 2}")
                    nc.scalar.activation(
                        et[:], xt_tiles[t][:], Act.Exp, scale=1.0 / T_KD,
                        accum_out=out_t[:, 40 + t : 41 + t],
                    )
                    ets.append(et)
                em = epool.tile([P, C_DIM], f32, tag="em")
                nc.scalar.activation(
                    em[:], psq[:], Act.Exp, scale=1.0 / T_KD,
                    accum_out=out_t[:, 44:45],
                )
                # tails of dots 2,3: Pool multiplies, ACT reduces
                split = i < N_TILES - 1
                prods = {}
                for t in (2, 3) if split else ():
                    pr = pdpool.tile([P, CSPL_N], f32, tag=f"pr{t}")
                    nc.gpsimd.tensor_tensor(
                        out=pr[:], in0=ets[t][:, CSPL:],
                        in1=dts[t][:, CSPL:], op=Alu.mult,
                    )
                    prods[t] = pr
                sink1 = spool.tile([P, C_DIM], f32, tag="sink")
                nc.scalar.activation(
                    sink1[:], st[:], Act.Exp, scale=1.0,
                    accum_out=out_t[:, 45:46],
                )
                sink2 = spool.tile([P, C_DIM], f32, tag="sink")
                nc.scalar.activation(
                    sink2[:], st[:], Act.Exp, scale=1.0 / T_KD,
                    accum_out=out_t[:, 46:47],
                )
                for t in (2, 3) if split else ():
                    snk = spool.tile([P, CSPL_N], f32, tag=f"ps{t}")
                    nc.scalar.activation(
                        snk[:], prods[t][:], Act.Copy, scale=1.0,
                        accum_out=out_t[:, 50 + t : 51 + t],
                    )

                # --- DVE: max8 fillers + dots in dependency-arrival order ---
                def dot(in0, in1, col, tag, n=C_DIM):
                    sc = scpool.tile([P, n], f32, tag=tag)
                    nc.vector.scalar_tensor_tensor(
                        out=sc[:], in0=in0, scalar=0.0, in1=in1,
                        op0=Alu.bypass, op1=Alu.mult,
                        accum_out=out_t[:, col : col + 1],
                    )

                nc.vector.max(out=out_t[:, 0:8], in_=xt_tiles[0][:])
                nc.vector.max(out=out_t[:, 8:16], in_=xt_tiles[1][:])
                dot(ets[0][:], dts[0][:], 47, "sc0")
                nc.vector.max(out=out_t[:, 16:24], in_=xt_tiles[2][:])
                dot(ets[1][:], dts[1][:], 48, "sc1")
                nc.vector.max(out=out_t[:, 24:32], in_=xt_tiles[3][:])
                dn = CSPL if split else C_DIM
                dot(ets[2][:, :dn], dts[2][:, :dn], 49, "sc0", dn)
                dot(ets[3][:, :dn], dts[3][:, :dn], 50, "sc1", dn)
                nc.vector.max(out=out_t[:, 32:40], in_=psq[:])
                dot(em[:], dmq[:], 51, "scm")

                nc.sync.dma_start(out=res[r0 : r0 + P, :], in_=out_t[:])

    nc.finalize()
    return nc


def _get_nc():
    if "nc" not in _CACHE:
        _CACHE["nc"] = _build_nc()
    return _CACHE["nc"]


def _run_device(in_maps, trace=False):
    from concourse.bass_utils import run_bass_kernel_spmd

    nc = _get_nc()
    return run_bass_kernel_spmd(
        nc, in_maps, core_ids=list(range(N_CORES)), trace=trace
    )


def _host_combine(res_cores, g, g_s):
    """res_cores: [N_CORES][B_LOC, OUT_COLS] f32; g: [B,4] gathered teacher
    logits (f64); g_s: [B] gathered student logits (f64)."""
    r = np.concatenate(res_cores, axis=0).astype(np.float64)
    # last tile of each core runs unsplit dots; its tail cols are unwritten
    last = (np.arange(B_FULL) % B_LOC) >= (B_LOC - P)
    r[last, 52:54] = 0.0

    g_m = g.mean(axis=1)                                     # mimic gathered
    gathered = np.concatenate([g, g_m[:, None]], axis=1)     # [B,5]

    m1 = r[:, [0, 8, 16, 24, 32]]
    m2 = r[:, [1, 9, 17, 25, 33]]
    S = r[:, 40:45]
    D = r[:, 47:52].copy()
    D[:, 2] += r[:, 52]
    D[:, 3] += r[:, 53]
    S1 = r[:, 45]
    S20 = r[:, 46]

    Cmin = g.min()
    shift = (-Cmin + EPS) if Cmin < 0 else 0.0

    margins = np.maximum(gathered - m2, 0.0)
    z = margins / T_THR
    z = z - z.max(axis=1, keepdims=True)
    ez = np.exp(z)
    thr = ez / ez.sum(axis=1, keepdims=True)

    max_preds = m1[:, :4].max() + shift

    lse_t = np.log(S)
    KD = T_KD * D / S + (T_KD * T_KD) * (np.log(S20)[:, None] - lse_t)
    CE = np.log(S1) - g_s

    w2 = (gathered + shift) / max_preds
    losses = (1.0 - w2) * CE[:, None] + w2 * KD
    return np.asarray((thr * losses).sum(axis=1).mean(), dtype=np.float32)


def kernel(outputs1, outputs2, outputs3, outputs4, out_s, targets,
           _trace=False, _return_results=False):
    xs = [np.ascontiguousarray(np.asarray(a, dtype=np.float32))
          for a in (outputs1, outputs2, outputs3, outputs4)]
    s = np.ascontiguousarray(np.asarray(out_s, dtype=np.float32))
    tg = np.asarray(targets).astype(np.int64)

    idx = np.arange(B_FULL)
    g = np.stack([x[idx, tg] for x in xs], axis=1).astype(np.float64)  # [B,4]
    g_s = s[idx, tg].astype(np.float64)

    ident = np.eye(P, dtype=np.float32)
    negid = (-np.eye(P, dtype=np.float32)).astype(np.float32)
    quarter = (0.25 * np.eye(P, dtype=np.float32)).astype(np.float32)
    in_maps = []
    for c in range(N_CORES):
        sl = slice(c * B_LOC, (c + 1) * B_LOC)
        m = {f"x{t}": xs[t][sl] for t in range(4)}
        m["s"] = s[sl]
        m["ident"] = ident
        m["negid"] = negid
        m["quarter"] = quarter
        in_maps.append(m)

    results = _run_device(in_maps, trace=_trace)
    res_cores = [results.results[c]["res"] for c in range(N_CORES)]
    out = _host_combine(res_cores, g, g_s)
    if _return_results:
        return out, results
    return out


# revision 22
# speedup vs baseline: 1.0874x; 1.0874x over previous
"""Trainium2 Bass kernel for the Dynamic MultiTeacher distillation loss.

Data-parallel over 8 NeuronCores, 1024 rows each, 8 row-tiles of 128.

Per tile [128, 1000] (all f32; PE inputs viewed as float32r):
  - PE (f32r identity matmuls; ~3.4x faster than fp32, rel err ~1.7e-4):
      d_0, d_1 = x_t - s                     (PSUM)
      psq      = 0.25*(x1+x2+x3+x4)          (PSUM, = mimic logits)
      dmq      = 0.25*(x1+x2+x3+x4) - s      (PSUM, = psq - s)
  - Pool: d_2, d_3 = x_t - s (SBUF out; Pool cannot touch PSUM and must not
    be loaded much beyond this - it is slow (~2.5us/pass) and extra work on
    it stalls the DVE dots that consume d_2/d_3).
  - ScalarE (7 exp passes, 1 elem/cycle): e_t = exp(x_t/20) + accum S_t
    (x4), e_m = exp(psq/20) + S_m, exp(s) + S1, exp(s/20) + S20 (sinks).
    Teacher exps first so DVE dots aren't starved; sinks last. Also 2 small
    Copy+accum reduces for the dot tails below.
  - VectorE (bottleneck, 10 passes): max8 x5 (4 teachers + mimic) and 5
    fused dots D_t = sum(e_t * d_t), D_m = sum(e_m * dmq) via
    scalar_tensor_tensor with full-tensor out (faster than broadcast out).
    The dots for t=2,3 cover only cols [0:576] on DVE; their tails are a
    Pool elementwise multiply + ScalarE Copy-with-accum, rebalancing ~1.5
    DVE passes onto the two less-loaded engines.
  - All input DMAs ride the sync queue (HWDGE triggers on Pool/Scalar cost
    those engines ~0.6us each).

Host (tiny O(B) work in f64): gathers x_t[i,t_i], global min/max scalars,
margins = relu(gathered - m2), KD_t = T*D_t/S_t + T^2*(lse_s - lse_t),
CE = lse1 - s_gathered, threshold softmax, w1/w2 blend, mean.
"""

import numpy as np

N_CORES = 8
B_FULL = 8192
C_DIM = 1000
B_LOC = B_FULL // N_CORES          # 1024 rows per core
P = 128                            # partitions
N_TILES = B_LOC // P               # 8 row-tiles per core

T_KD = 20.0
T_THR = 6.0
EPS = 1e-05

# device output column layout: [P, 52]
#   cols 8t..8t+7 : top8 of teacher t (t=0..3 real, t=4 mimic psq)
#   col 40+t      : S_t = sum exp(x_t/20)   (t=4 mimic)
#   col 45        : S1  = sum exp(s)
#   col 46        : S20 = sum exp(s/20)
#   col 47+t      : D_t = sum e_t*(x_t - s) (t=4: D_m = sum e_m*dmq)
#   col 52/53     : partial dot tails for t=2/3 over cols 704:1000
OUT_COLS = 54

_CACHE = {}


def _build_nc():
    import concourse.bacc as bacc
    import concourse.mybir as mybir
    from concourse import tile

    nc = bacc.Bacc(
        "TRN2",
        target_bir_lowering=False,
        debug=False,
        num_devices=N_CORES,
    )
    f32 = mybir.dt.float32
    f32r = mybir.dt.float32r
    Alu = mybir.AluOpType
    Act = mybir.ActivationFunctionType

    xs = [
        nc.dram_tensor(f"x{t}", [B_LOC, C_DIM], f32, kind="ExternalInput").ap()
        for t in range(4)
    ]
    s_dram = nc.dram_tensor("s", [B_LOC, C_DIM], f32, kind="ExternalInput").ap()
    ident = nc.dram_tensor("ident", [P, P], f32, kind="ExternalInput").ap()
    negid = nc.dram_tensor("negid", [P, P], f32, kind="ExternalInput").ap()
    quarter = nc.dram_tensor("quarter", [P, P], f32, kind="ExternalInput").ap()
    res = nc.dram_tensor("res", [B_LOC, OUT_COLS], f32, kind="ExternalOutput").ap()

    HALVES = ((0, 512), (512, C_DIM))
    CSPL = 640
    CSPL_N = C_DIM - CSPL

    with tile.TileContext(nc) as tc:
        with (
            tc.tile_pool(name="const", bufs=1) as cpool,
            tc.tile_pool(name="io", bufs=3) as xpool,
            tc.tile_pool(name="exps", bufs=2) as epool,
            tc.tile_pool(name="pdif", bufs=2) as pdpool,
            tc.tile_pool(name="sink", bufs=2) as spool,
            tc.tile_pool(name="sc", bufs=2) as scpool,
            tc.tile_pool(name="outs", bufs=2) as opool,
            tc.tile_pool(name="psd", bufs=1, space="PSUM") as psdpool,
            tc.tile_pool(name="psq", bufs=1, space="PSUM") as psqpool,
            tc.tile_pool(name="psm", bufs=1, space="PSUM") as psmpool,
        ):
            # f32r weight tiles; triggered from the scalar queue so the
            # sync/gpsimd queues start streaming x/s immediately
            id_t = cpool.tile([P, P], f32r, tag="id")
            nc.scalar.dma_start(out=id_t[:], in_=ident.bitcast(f32r))
            nid_t = cpool.tile([P, P], f32r, tag="nid")
            nc.scalar.dma_start(out=nid_t[:], in_=negid.bitcast(f32r))
            q_t = cpool.tile([P, P], f32r, tag="q")
            nc.scalar.dma_start(out=q_t[:], in_=quarter.bitcast(f32r))

            for i in range(N_TILES):
                r0 = i * P
                st = xpool.tile([P, C_DIM], f32, tag="s")
                nc.sync.dma_start(
                    out=st[:].bitcast(f32r),
                    in_=s_dram[r0 : r0 + P, :].bitcast(f32r),
                )
                xt_tiles = []
                for t in range(4):
                    xt = xpool.tile([P, C_DIM], f32, tag=f"x{t}")
                    dma_eng = nc.sync
                    dma_eng.dma_start(
                        out=xt[:].bitcast(f32r),
                        in_=xs[t][r0 : r0 + P, :].bitcast(f32r),
                    )
                    xt_tiles.append(xt)

                out_t = opool.tile([P, OUT_COLS], f32)

                # --- PE: d0, d1 (PSUM), psq, dmq ---
                dts = [None] * 4
                for t in (0, 1):
                    dt = psdpool.tile([P, C_DIM], f32, tag=f"d{t}")
                    for c0, c1 in HALVES:
                        nc.tensor.matmul(
                            dt[:, c0:c1], id_t[:],
                            xt_tiles[t][:, c0:c1].bitcast(f32r),
                            start=True, stop=False,
                        )
                        nc.tensor.matmul(
                            dt[:, c0:c1], nid_t[:],
                            st[:, c0:c1].bitcast(f32r),
                            start=False, stop=True,
                        )
                    dts[t] = dt

                psq = psqpool.tile([P, C_DIM], f32, tag="q")
                for c0, c1 in HALVES:
                    for t in range(4):
                        nc.tensor.matmul(
                            psq[:, c0:c1], q_t[:],
                            xt_tiles[t][:, c0:c1].bitcast(f32r),
                            start=(t == 0), stop=(t == 3),
                        )
                dmq = psmpool.tile([P, C_DIM], f32, tag="m")
                for c0, c1 in HALVES:
                    for t in range(4):
                        nc.tensor.matmul(
                            dmq[:, c0:c1], q_t[:],
                            xt_tiles[t][:, c0:c1].bitcast(f32r),
                            start=(t == 0), stop=False,
                        )
                    nc.tensor.matmul(
                        dmq[:, c0:c1], nid_t[:],
                        st[:, c0:c1].bitcast(f32r),
                        start=False, stop=True,
                    )

                # --- Pool: d2, d3 (SBUF out) ---
                for t in (2, 3):
                    dt = pdpool.tile([P, C_DIM], f32, tag=f"d{t}")
                    nc.gpsimd.tensor_tensor(
                        out=dt[:], in0=xt_tiles[t][:], in1=st[:],
                        op=Alu.subtract,
                    )
                    dts[t] = dt

                # --- ACT: teacher exps first, mimic, sinks last ---
                ets = []
                for t in range(4):
                    et = epool.tile([P, C_DIM], f32, tag=f"e{t % 2}")
                    nc.scalar.activation(
                        et[:], xt_tiles[t][:], Act.Exp, scale=1.0 / T_KD,
                        accum_out=out_t[:, 40 + t : 41 + t],
                    )
                    ets.append(et)
                em = epool.tile([P, C_DIM], f32, tag="em")
                nc.scalar.activation(
                    em[:], psq[:], Act.Exp, scale=1.0 / T_KD,
                    accum_out=out_t[:, 44:45],
                )
                # tails of dots 2,3: Pool multiplies, ACT reduces
                prods = {}
                for t in (2, 3):
                    pr = pdpool.tile([P, CSPL_N], f32, tag=f"pr{t}")
                    nc.gpsimd.tensor_tensor(
                        out=pr[:], in0=ets[t][:, CSPL:],
                        in1=dts[t][:, CSPL:], op=Alu.mult,
                    )
                    prods[t] = pr
                sink1 = spool.tile([P, C_DIM], f32, tag="sink")
                nc.scalar.activation(
                    sink1[:], st[:], Act.Exp, scale=1.0,
                    accum_out=out_t[:, 45:46],
                )
                sink2 = spool.tile([P, C_DIM], f32, tag="sink")
                nc.scalar.activation(
                    sink2[:], st[:], Act.Exp, scale=1.0 / T_KD,
                    accum_out=out_t[:, 46:47],
                )
                for t in (2, 3):
                    snk = spool.tile([P, CSPL_N], f32, tag=f"ps{t}")
                    nc.scalar.activation(
                        snk[:], prods[t][:], Act.Copy, scale=1.0,
                        accum_out=out_t[:, 50 + t : 51 + t],
                    )

                # --- DVE: max8 fillers + dots in dependency-arrival order ---
                def dot(in0, in1, col, tag, n=C_DIM):
                    sc = scpool.tile([P, n], f32, tag=tag)
                    nc.vector.scalar_tensor_tensor(
                        out=sc[:], in0=in0, scalar=0.0, in1=in1,
                        op0=Alu.bypass, op1=Alu.mult,
                        accum_out=out_t[:, col : col + 1],
                    )

                nc.vector.max(out=out_t[:, 0:8], in_=xt_tiles[0][:])
                nc.vector.max(out=out_t[:, 8:16], in_=xt_tiles[1][:])
                dot(ets[0][:], dts[0][:], 47, "sc0")
                nc.vector.max(out=out_t[:, 16:24], in_=xt_tiles[2][:])
                dot(ets[1][:], dts[1][:], 48, "sc1")
                nc.vector.max(out=out_t[:, 24:32], in_=xt_tiles[3][:])
                dot(ets[2][:, :CSPL], dts[2][:, :CSPL], 49, "sc0", CSPL)
                dot(ets[3][:, :CSPL], dts[3][:, :CSPL], 50, "sc1", CSPL)
                nc.vector.max(out=out_t[:, 32:40], in_=psq[:])
                dot(em[:], dmq[:], 51, "scm")

                nc.sync.dma_start(out=res[r0 : r0 + P, :], in_=out_t[:])

    nc.finalize()
    return nc


def _get_nc():
    if "nc" not in _CACHE:
        _CACHE["nc"] = _build_nc()
    return _CACHE["nc"]


def _run_device(in_maps, trace=False):
    from concourse.bass_utils import run_bass_kernel_spmd

    nc = _get_nc()
    return run_bass_kernel_spmd(
        nc, in_maps, core_ids=list(range(N_CORES)), trace=trace
    )


def _host_combine(res_cores, g, g_s):
    """res_cores: [N_CORES][B_LOC, OUT_COLS] f32; g: [B,4] gathered teacher
    logits (f64); g_s: [B] gathered student logits (f64)."""
    r = np.concatenate(res_cores, axis=0).astype(np.float64)  # [B, 52]

    g_m = g.mean(axis=1)                                     # mimic gathered
    gathered = np.concatenate([g, g_m[:, None]], axis=1)     # [B,5]

    m1 = r[:, [0, 8, 16, 24, 32]]
    m2 = r[:, [1, 9, 17, 25, 33]]
    S = r[:, 40:45]
    D = r[:, 47:52].copy()
    D[:, 2] += r[:, 52]
    D[:, 3] += r[:, 53]
    S1 = r[:, 45]
    S20 = r[:, 46]

    Cmin = g.min()
    shift = (-Cmin + EPS) if Cmin < 0 else 0.0

    margins = np.maximum(gathered - m2, 0.0)
    z = margins / T_THR
    z = z - z.max(axis=1, keepdims=True)
    ez = np.exp(z)
    thr = ez / ez.sum(axis=1, keepdims=True)

    max_preds = m1[:, :4].max() + shift

    lse_t = np.log(S)
    KD = T_KD * D / S + (T_KD * T_KD) * (np.log(S20)[:, None] - lse_t)
    CE = np.log(S1) - g_s

    w2 = (gathered + shift) / max_preds
    losses = (1.0 - w2) * CE[:, None] + w2 * KD
    return np.asarray((thr * losses).sum(axis=1).mean(), dtype=np.float32)


def kernel(outputs1, outputs2, outputs3, outputs4, out_s, targets,
           _trace=False, _return_results=False):
    xs = [np.ascontiguousarray(np.asarray(a, dtype=np.float32))
          for a in (outputs1, outputs2, outputs3, outputs4)]
    s = np.ascontiguousarray(np.asarray(out_s, dtype=np.float32))
    tg = np.asarray(targets).astype(np.int64)

    idx = np.arange(B_FULL)
    g = np.stack([x[idx, tg] for x in xs], axis=1).astype(np.float64)  # [B,4]
    g_s = s[idx, tg].astype(np.float64)

    ident = np.eye(P, dtype=np.float32)
    negid = (-np.eye(P, dtype=np.float32)).astype(np.float32)
    quarter = (0.25 * np.eye(P, dtype=np.float32)).astype(np.float32)
    in_maps = []
    for c in range(N_CORES):
        sl = slice(c * B_LOC, (c + 1) * B_LOC)
        m = {f"x{t}": xs[t][sl] for t in range(4)}
        m["s"] = s[sl]
        m["ident"] = ident
        m["negid"] = negid
        m["quarter"] = quarter
        in_maps.append(m)

    results = _run_device(in_maps, trace=_trace)
    res_cores = [results.results[c]["res"] for c in range(N_CORES)]
    out = _host_combine(res_cores, g, g_s)
    if _return_results:
        return out, results
    return out
